# revision 2
# baseline (speedup 1.0000x reference)
"""DynamicLinear (MoE routing) Trainium2 Bass kernel.

Math (per sample b):
    out[b] = sum_k attn[b,k] * (x[b] @ W[k].T + bias[k])
           = sum_k attn[b,k] * (x[b] @ W[k].T) + attn[b] @ bias

Sharding: 8 cores in a 2x4 grid over (batch, out_features).
Each core computes out[b_half, o_quarter] from x[b_half] and
W[:, o_quarter, :] -- no cross-core communication.

The host ships x and W pre-tiled and pre-cast to bf16 in the exact
SBUF layouts the kernel consumes (contraction dim on partitions,
contiguous per partition), so every load is a plain full-rate HWDGE
DMA. Matmuls run bf16 x bf16 with fp32 PSUM accumulation -- the
compute roofline for this problem (fp8 DoubleRow is 2x rate but any
split-precision scheme that passes the accuracy bar needs >=3 terms,
a net loss).

Schedule (v2, from baseline trace analysis):
  * HAM pre-warm: DVE memsets a scratch bf16 tile, then ~18 dummy
    N=512 matmuls with no DMA deps run while the DMA rings ramp, so
    the PE clock-gate (4/8 -> 8/8 at ~3.4us of busy) is already warm
    when real data lands (baseline ran cold until 26us).
  * Critical-path DMAs: attn (32 KiB) first, then expert-0 weights
    as 16 x 128 KiB ii-slabs (baseline's 4 x 0.5 MiB granules made
    the first psum group wait ~7us for all of expert 0).
  * bias broadcast via HWDGE on sync (baseline used gpsimd SWDGE for
    the 1 MiB replicated read -> landed at ~42us, nearly stalling
    the 8-deep psum ring).
  * Sweep 0 runs in blocks of 4 batch-tiles interleaved ii-major, so
    during the DMA ramp any arrived (slab, x-tile) pair is runnable
    work and one late tile doesn't idle the PE.
  * Sweeps 1..3 per-tile groups; acc init (attn-weighted bias) is
    prefetched on DVE one block ahead; out stores at k=3; the final
    tile's combine+store is split in halves across two queues to
    shorten the tail.
"""

import numpy as np

_B, _K, _IN, _OUT = 4096, 4, 2048, 2048
_GRID_B, _GRID_O = 2, 4
_BL = _B // _GRID_B      # 2048 batch rows per core
_OL = _OUT // _GRID_O    # 512 out cols per core
_NBT = _BL // 128        # 16 b tiles
_NIT = _IN // 128        # 16 contraction tiles
_NH = _NIT // 4          # ii-tiles per W load granule (experts 1..3)
_BLK = 4                 # sweep-0 tile block (psum ring is 8 banks)
_WARMUP_MMS = 18

_CACHE = {}
LAST_RESULTS = None


def _build_program():
    import concourse.bass as bass
    import concourse.tile as tile
    from concourse import bacc, mybir

    f32 = mybir.dt.float32
    bf16 = mybir.dt.bfloat16
    MULT = mybir.AluOpType.mult
    ADD = mybir.AluOpType.add

    nc = bacc.Bacc("TRN2", target_bir_lowering=False, debug=False)
    xT = nc.dram_tensor("xT", [_NBT, 128, _NIT, 128], bf16,
                        kind="ExternalInput").ap()
    attn = nc.dram_tensor("attn", [_BL, _K], f32, kind="ExternalInput").ap()
    wT = nc.dram_tensor("wT", [_K, 128, _NIT, _OL], bf16,
                        kind="ExternalInput").ap()
    bias = nc.dram_tensor("bias", [_K, _OL], f32, kind="ExternalInput").ap()
    out = nc.dram_tensor("out", [_BL, _OL], f32, kind="ExternalOutput").ap()

    with tile.TileContext(nc) as tc:
        with (
            tc.tile_pool(name="w0", bufs=_NIT) as w0p,
            tc.tile_pool(name="wt", bufs=4 * (_K - 1)) as wtp,
            tc.tile_pool(name="xt", bufs=_NBT) as xtp,
            tc.tile_pool(name="singles", bufs=1) as singles,
            tc.tile_pool(name="acc", bufs=_NBT) as accp,
            tc.tile_pool(name="psum", bufs=8, space="PSUM") as psump,
        ):
            # --- HAM pre-warm: no DMA deps, runs during ring startup ---
            warm = singles.tile([128, _OL], bf16, tag="warm")
            nc.vector.memset(warm, 0.0)
            for i in range(_WARMUP_MMS):
                wps = psump.tile([128, _OL], f32, tag="ps",
                                 name=f"warmps{i}")
                nc.tensor.matmul(wps, lhsT=warm[:, 0:128], rhs=warm,
                                 start=True, stop=True)

            # --- critical-path loads ---
            # attn for all b_tiles, b on partitions:
            # attn_sb[p, t, k] = attn[t*128 + p, k]
            attn_sb = singles.tile([128, _NBT, _K], f32, tag="attn")
            attn_src = bass.AP(
                tensor=attn.tensor,
                offset=attn.offset,
                ap=[[_K, 128], [128 * _K, _NBT], [1, _K]],
            )
            nc.sync.dma_start(out=attn_sb, in_=attn_src)

            # expert-0 weights in 128 KiB ii-slabs (just-in-time ramp):
            # w0[ii][i_in, o] = W[0][o, ii*128 + i_in]
            w0 = []
            for ii in range(_NIT):
                t_ = w0p.tile([128, _OL], bf16, tag="w0", name=f"w0_{ii}")
                nc.sync.dma_start(out=t_, in_=wT[0, :, ii])
                w0.append(t_)

            # x stream on scalar queue:
            # xt[t][i_in, ii, b] = x[t*128 + b, ii*128 + i_in]
            xts = []
            for t in range(_NBT):
                t_ = xtp.tile([128, _NIT, 128], bf16, tag="xt",
                              name=f"xt{t}")
                nc.scalar.dma_start(out=t_, in_=xT[t])
                xts.append(t_)

            # bias replicated across all 128 partitions (HWDGE broadcast)
            bias_rep = singles.tile([128, _K, _OL], f32, tag="bias")
            nc.sync.dma_start(
                out=bias_rep,
                in_=bass.AP(
                    tensor=bias.tensor,
                    offset=bias.offset,
                    ap=[[0, 128], bias.ap[0], bias.ap[1]],
                ),
            )

            # experts 1..3 in 0.5 MiB granules:
            # wt[k][h][i_in, j, o] = W[k][o, (h*NH + j)*128 + i_in]
            wt = {}
            for k in range(1, _K):
                for h in range(_NIT // _NH):
                    t_ = wtp.tile([128, _NH, _OL], bf16, tag="wt",
                                  name=f"wt{k}_{h}")
                    nc.sync.dma_start(out=t_,
                                      in_=wT[k, :, h * _NH:(h + 1) * _NH])
                    wt[(k, h)] = t_

            acc = [None] * _NBT

            def init_acc(t):
                # acc[t] = sum_k attn[:,k] * bias[k]  (DVE, one block ahead)
                at = accp.tile([128, _OL], f32, tag="acc", name=f"acc{t}")
                acc[t] = at
                a_sc = attn_sb[:, t, :]
                nc.vector.tensor_scalar(
                    out=at, in0=bias_rep[:, 0, :],
                    scalar1=a_sc[:, 0:1], scalar2=None, op0=MULT,
                )
                for kk in range(1, _K):
                    nc.vector.scalar_tensor_tensor(
                        out=at, in0=bias_rep[:, kk, :],
                        scalar=a_sc[:, kk:kk + 1], in1=at,
                        op0=MULT, op1=ADD,
                    )

            def combine(t, k, ps, lo=0, hi=_OL):
                # acc[t][:, lo:hi] += attn[:,k] * ps[:, lo:hi]
                nc.vector.scalar_tensor_tensor(
                    out=acc[t][:, lo:hi], in0=ps[:, lo:hi],
                    scalar=attn_sb[:, t, k:k + 1], in1=acc[t][:, lo:hi],
                    op0=MULT, op1=ADD,
                )

            # --- sweep k=0: blocks of 4 tiles, ii-major interleave ---
            for blk in range(_NBT // _BLK):
                ts = range(blk * _BLK, (blk + 1) * _BLK)
                for t in ts:
                    init_acc(t)
                pss = {t: psump.tile([128, _OL], f32, tag="ps",
                                     name=f"ps0_{t}")
                       for t in ts}
                for ii in range(_NIT):
                    for t in ts:
                        nc.tensor.matmul(
                            pss[t],
                            lhsT=xts[t][:, ii, :],
                            rhs=w0[ii],
                            start=(ii == 0), stop=(ii == _NIT - 1),
                        )
                for t in ts:
                    combine(t, 0, pss[t])

            # --- sweeps k=1..3: per-tile groups ---
            for k in range(1, _K):
                for t in range(_NBT):
                    ps = psump.tile([128, _OL], f32, tag="ps",
                                    name=f"ps{k}_{t}")
                    for ii in range(_NIT):
                        nc.tensor.matmul(
                            ps,
                            lhsT=xts[t][:, ii, :],
                            rhs=wt[(k, ii // _NH)][:, ii % _NH, :],
                            start=(ii == 0), stop=(ii == _NIT - 1),
                        )
                    if k < _K - 1:
                        combine(t, k, ps)
                    elif t < _NBT - 1:
                        combine(t, k, ps)
                        nc.sync.dma_start(
                            out=out[t * 128:(t + 1) * 128, :],
                            in_=acc[t],
                        )
                    else:
                        # final tile: halves on two queues to cut the tail
                        h = _OL // 2
                        combine(t, k, ps, 0, h)
                        nc.sync.dma_start(
                            out=out[t * 128:(t + 1) * 128, 0:h],
                            in_=acc[t][:, 0:h],
                        )
                        combine(t, k, ps, h, _OL)
                        nc.scalar.dma_start(
                            out=out[t * 128:(t + 1) * 128, h:_OL],
                            in_=acc[t][:, h:_OL],
                        )

    nc.compile()
    return nc


def _get_program():
    if "nc" not in _CACHE:
        _CACHE["nc"] = _build_program()
    return _CACHE["nc"]


def _ensure_axon_hooks_importable():
    """bass_utils' trace branch imports antenv.axon_hooks, which the
    trimmed agent image may lack; stub it (hook=None) so a stray
    BASS_TRACE=1 degrades to an untraced run instead of crashing."""
    import sys
    import types

    try:
        import antenv.axon_hooks  # noqa: F401
        return
    except ImportError:
        pass
    mod = types.ModuleType("antenv.axon_hooks")
    mod._hook = None
    mod.get_axon_ntff_profile_hook = lambda: mod._hook

    def _set(h):
        mod._hook = h

    mod.set_axon_ntff_profile_hook = _set
    sys.modules["antenv.axon_hooks"] = mod
    try:
        import antenv
        antenv.axon_hooks = mod
    except ImportError:
        pass


def kernel(**inputs):
    global LAST_RESULTS
    from concourse.bass_utils import run_bass_kernel_spmd

    _ensure_axon_hooks_importable()

    x = np.ascontiguousarray(inputs["x"], dtype=np.float32)
    attn = np.ascontiguousarray(inputs["softmax_attention"], dtype=np.float32)
    w = np.ascontiguousarray(inputs["weight"], dtype=np.float32)
    b = np.ascontiguousarray(inputs["bias"], dtype=np.float32)

    nc = _get_program()
    in_maps = []
    for c in range(8):
        gb, go = divmod(c, _GRID_O)
        x_sl = x[gb * _BL:(gb + 1) * _BL]
        w_sl = w[:, go * _OL:(go + 1) * _OL, :]
        # tile-contiguous device layouts (see _build_program):
        # xT[t, i_in, ii, b_in] = x[t*128 + b_in, ii*128 + i_in]
        # wT[k, i_in, ii, o]    = W[k, o, ii*128 + i_in]
        import ml_dtypes
        xT = np.ascontiguousarray(
            x_sl.T.reshape(_NIT, 128, _NBT, 128).transpose(2, 1, 0, 3)
        ).astype(ml_dtypes.bfloat16)
        wTa = np.ascontiguousarray(
            w_sl.transpose(0, 2, 1)
            .reshape(_K, _NIT, 128, _OL).transpose(0, 2, 1, 3)
        ).astype(ml_dtypes.bfloat16)
        in_maps.append({
            "xT": xT,
            "attn": np.ascontiguousarray(attn[gb * _BL:(gb + 1) * _BL]),
            "wT": wTa,
            "bias": np.ascontiguousarray(b[:, go * _OL:(go + 1) * _OL]),
        })

    res = run_bass_kernel_spmd(nc, in_maps, list(range(8)))
    LAST_RESULTS = res

    full = np.empty((_B, _OUT), dtype=np.float32)
    for c in range(8):
        gb, go = divmod(c, _GRID_O)
        full[gb * _BL:(gb + 1) * _BL, go * _OL:(go + 1) * _OL] = \
            res.results[c]["out"]
    return full
